# revision 1
# baseline (speedup 1.0000x reference)
"""DenseMPNN Trainium2 kernel (8-core SPMD, batch data-parallel).

Strategy:
- Shard batch B=32 across 8 cores (4 molecules/core); replicate weights.
- Host packs each molecule's ~4%-dense adjacency into an UNDIRECTED edge
  list (E_u <= 128): partition row e holds both directions of undirected
  edge {v,w} as two 256-wide feature halves (fwd = v->w, bwd = w->v).
  The reverse-edge lookup in the MPNN message then costs nothing: it is
  the other half of the same row.
    H0[e,(f|b)] = relu(bonds_e @ Wi_bond + atom_part[src])
    iter:  HWh = H @ Wh                  (via PE transpose + matmul)
           P = Tf^T@HWh_f + Tb^T@HWh_b  (= agg @ Wh, [N,H])
           Q_f = G1f@P - HWh_b ; Q_b = G1b@P - HWh_f   (= msg @ Wh)
           H_f = relu(H0_f + inv_f * Q_f) ; H_b likewise
    out = relu(atoms@Wo_a + (agg_final)@Wo_h + bo)
  Gather/scatter matrices (T*, G1*) are host-built one-hot operands so the
  device does only matmuls + elementwise; all state stays in SBUF.
"""

import numpy as np

_B, _N, _A, _EB, _H = 32, 64, 133, 14, 256
_DEPTH = 3
_NCORES = 8
_MPC = _B // _NCORES  # molecules per core

_cache = {}
# float32r streams fp32 data through the PE at full rate (1 cyc/row vs 4 for
# strict fp32 when the moving free-dim is >=256). Measured end-to-end relative
# error vs the fp32 reference: 1.7e-4 (residual variance ~1e-8) — inside any
# scale-relative tolerance; strict fp32 is ~2x slower end-to-end.
_DTYPE = "float32r"


def _build_nc(E_u, dtype_name="float32", reps=1):
    import sys
    for p in ("/opt/trn_rl_repo",):
        if p not in sys.path:
            sys.path.insert(0, p)
    import concourse.bass as bass  # noqa: F401
    import concourse.mybir as mybir
    import concourse.tile as tile
    from concourse import bacc
    from concourse.masks import make_identity

    FD = getattr(mybir.dt, dtype_name)
    F32 = mybir.dt.float32
    HT_N = _H // 128  # hidden chunks of 128
    RELU = mybir.ActivationFunctionType.Relu
    MULT = mybir.AluOpType.mult
    ADD = mybir.AluOpType.add

    nc = bacc.Bacc(None, target_bir_lowering=False, debug=False)

    # --- I/O --- (dram dtype matches compute dtype; float32r is f32 bits)
    # All per-molecule operands are packed into one [128, META_COLS] page so a
    # single DMA loads a molecule: atomsT | g1 | tm | inv | X-feature bands,
    # where X[:, d, e] = [atoms[src(e,d)] ; bonds(e,d)] (147 rows split 128+19)
    # so H0 = X^T @ Wi directly.
    KX = _A + _EB  # 147
    META_COLS = 64 + 64 + 2 * E_u + 2 * _N + 2 + 2 * E_u + 2 * E_u
    meta = nc.dram_tensor("meta", [_MPC, 128, META_COLS], FD, kind="ExternalInput")
    Wi = nc.dram_tensor("Wi", [KX, _H], FD, kind="ExternalInput")
    WoA = nc.dram_tensor("WoA", [_A + 1, _H], FD, kind="ExternalInput")
    Wh = nc.dram_tensor("Wh", [_H, _H], FD, kind="ExternalInput")
    WoH = nc.dram_tensor("WoH", [_H, _H], FD, kind="ExternalInput")
    out = nc.dram_tensor("out", [_MPC, _N, _H], F32, kind="ExternalOutput")

    with tile.TileContext(nc) as tc:
        import contextlib
        with contextlib.ExitStack() as ctx:
            consts = ctx.enter_context(tc.tile_pool(name="consts", bufs=1))
            inp = ctx.enter_context(tc.tile_pool(name="inp", bufs=4))
            work = ctx.enter_context(tc.tile_pool(name="work", bufs=4))
            hbuf = ctx.enter_context(tc.tile_pool(name="hbuf", bufs=4))
            obuf = ctx.enter_context(tc.tile_pool(name="obuf", bufs=2))
            ps_mm = ctx.enter_context(tc.tile_pool(name="ps_mm", bufs=3, space="PSUM"))
            ps_tr = ctx.enter_context(tc.tile_pool(name="ps_tr", bufs=2, space="PSUM"))
            ps_sm = ctx.enter_context(tc.tile_pool(name="ps_sm", bufs=3, space="PSUM"))

            # ---- constants (loaded once) ----
            if FD == F32:
                ident = consts.tile([128, 128], FD)
                make_identity(nc, ident)
            else:
                ident_f32 = consts.tile([128, 128], F32)
                make_identity(nc, ident_f32)
                ident = consts.tile([128, 128], FD)
                nc.vector.tensor_copy(out=ident, in_=ident_f32)
            negident = consts.tile([128, 128], FD)
            nc.vector.tensor_scalar_mul(out=negident, in0=ident, scalar1=-1.0)
            wi1_s = consts.tile([128, _H], FD)
            nc.sync.dma_start(out=wi1_s, in_=Wi[0:128, :])
            wi2_s = consts.tile([KX - 128, _H], FD)
            nc.sync.dma_start(out=wi2_s, in_=Wi[128:, :])
            woa1_s = consts.tile([128, _H], FD)
            nc.sync.dma_start(out=woa1_s, in_=WoA[0:128, :])
            woa2_s = consts.tile([_A + 1 - 128, _H], FD)
            nc.sync.dma_start(out=woa2_s, in_=WoA[128:, :])
            wh_s = consts.tile([128, HT_N, _H], FD)
            nc.sync.dma_start(out=wh_s, in_=Wh.rearrange("(c p) n -> p c n", p=128))
            woh_s = consts.tile([128, HT_N, _H], FD)
            nc.sync.dma_start(out=woh_s, in_=WoH.rearrange("(c p) n -> p c n", p=128))

            # Phase-lockstep emission: run each phase for ALL molecules before
            # the next phase, so cross-engine latency amortizes across the
            # 4-molecule pipeline instead of serializing one molecule's chain.
            for rep in range(reps):
                S = [{} for _ in range(_MPC)]
                for m in range(_MPC):
                    mt = inp.tile([128, META_COLS], FD, tag="meta", name=f"mt{m}")
                    # X+inv band is last in the layout but needed first (H0):
                    # split the load so compute starts before gather bands land.
                    xcols = 4 * E_u + 2
                    nc.sync.dma_start(out=mt[:, META_COLS - xcols:],
                                      in_=meta[m, :, META_COLS - xcols:])
                    nc.sync.dma_start(out=mt[:, 0:META_COLS - xcols],
                                      in_=meta[m, :, 0:META_COLS - xcols])
                    c0 = 0
                    s = S[m]
                    s["aT1"] = mt[:, c0:c0 + _N]; c0 += _N
                    s["aT2"] = mt[0:_A + 1 - 128, c0:c0 + _N]; c0 += _N
                    s["g1"] = mt[0:_N, c0:c0 + 2 * E_u].rearrange(
                        "p (d e) -> p d e", d=2); c0 += 2 * E_u
                    s["tm"] = mt[0:E_u, c0:c0 + 2 * _N].rearrange(
                        "p (d n) -> p d n", d=2); c0 += 2 * _N
                    s["inv"] = mt[0:E_u, c0:c0 + 2]; c0 += 2
                    s["X1"] = mt[:, c0:c0 + 2 * E_u].rearrange(
                        "p (d e) -> p d e", d=2); c0 += 2 * E_u
                    s["X2"] = mt[0:KX - 128, c0:c0 + 2 * E_u].rearrange(
                        "p (d e) -> p d e", d=2); c0 += 2 * E_u

                # H0 = X^T @ Wi   [E_u, 2, H]
                for m in range(_MPC):
                    ps_h0 = ps_mm.tile([E_u, 2, _H], F32, tag="mm", name=f"psh0{m}")
                    for d in range(2):
                        nc.tensor.matmul(ps_h0[:, d, :], S[m]["X1"][:, d, :], wi1_s,
                                         start=True, stop=False)
                        nc.tensor.matmul(ps_h0[:, d, :], S[m]["X2"][:, d, :], wi2_s,
                                         start=False, stop=True)
                    S[m]["ps_h0"] = ps_h0
                for m in range(_MPC):
                    h0 = hbuf.tile([E_u, 2, _H], FD, tag="h0", name=f"h0_{m}")
                    nc.scalar.activation(out=h0, in_=S[m]["ps_h0"], func=RELU)
                    S[m]["h0"] = h0
                    S[m]["h"] = h0  # initial H == H0 (mask folded into packing)

                # message passing iterations
                for it in range(_DEPTH - 1):
                    for m in range(_MPC):
                        ps_t = ps_tr.tile([128, HT_N, 2, E_u], FD, tag="tr",
                                          name=f"pst{m}")
                        h = S[m]["h"]
                        for hh in range(HT_N):
                            for d in range(2):
                                nc.tensor.transpose(
                                    ps_t[:, hh, d, :],
                                    h[:, d, hh * 128:(hh + 1) * 128],
                                    ident[:E_u, :E_u])
                        S[m]["ps_t"] = ps_t
                    for m in range(_MPC):
                        ht_all = work.tile([128, HT_N, 2, E_u], FD, tag="ht",
                                           name=f"ht{m}")
                        (nc.scalar.copy if m % 2 else nc.vector.tensor_copy)(
                            out=ht_all, in_=S[m]["ps_t"])
                        S[m]["ht"] = ht_all
                    for m in range(_MPC):
                        ps_hw = ps_mm.tile([E_u, 2, _H], F32, tag="mm",
                                           name=f"pshw{m}")
                        for d in range(2):
                            for hh in range(HT_N):
                                nc.tensor.matmul(ps_hw[:, d, :],
                                                 S[m]["ht"][:, hh, d, :],
                                                 wh_s[:, hh, :],
                                                 start=(hh == 0),
                                                 stop=(hh == HT_N - 1))
                        S[m]["ps_hw"] = ps_hw
                    for m in range(_MPC):
                        hwh = work.tile([E_u, 2, _H], FD, tag="hwh", name=f"hwh{m}")
                        nc.vector.tensor_copy(out=hwh, in_=S[m]["ps_hw"])
                        S[m]["hwh"] = hwh
                    for m in range(_MPC):
                        ps_p = ps_sm.tile([_N, _H], F32, tag="sm", name=f"psp{m}")
                        for d in range(2):
                            nc.tensor.matmul(ps_p, S[m]["tm"][:, d, :],
                                             S[m]["hwh"][:, d, :],
                                             start=(d == 0), stop=(d == 1))
                        S[m]["ps_p"] = ps_p
                    for m in range(_MPC):
                        p_s = work.tile([_N, _H], FD, tag="p", name=f"p{m}")
                        nc.scalar.copy(out=p_s, in_=S[m]["ps_p"])
                        S[m]["p"] = p_s
                    for m in range(_MPC):
                        ps_q = ps_mm.tile([E_u, 2, _H], F32, tag="mm",
                                          name=f"psq{m}")
                        for d in range(2):
                            nc.tensor.matmul(ps_q[:, d, :], S[m]["g1"][:, d, :],
                                             S[m]["p"], start=True, stop=False)
                            nc.tensor.matmul(ps_q[:, d, :], negident[:E_u, :E_u],
                                             S[m]["hwh"][:, 1 - d, :],
                                             start=False, stop=True)
                        S[m]["ps_q"] = ps_q
                    for m in range(_MPC):
                        hn_pre = work.tile([E_u, 2, _H], FD, tag="hn_pre",
                                           name=f"hnp{m}")
                        for d in range(2):
                            nc.vector.scalar_tensor_tensor(
                                out=hn_pre[:, d, :], in0=S[m]["ps_q"][:, d, :],
                                scalar=S[m]["inv"][:, d:d + 1],
                                in1=S[m]["h0"][:, d, :], op0=MULT, op1=ADD)
                        S[m]["hn_pre"] = hn_pre
                    for m in range(_MPC):
                        hn = hbuf.tile([E_u, 2, _H], FD, tag="hn", name=f"hn{m}")
                        nc.scalar.activation(out=hn, in_=S[m]["hn_pre"], func=RELU)
                        S[m]["h"] = hn

                # readout
                for m in range(_MPC):
                    ps_a = ps_tr.tile([128, HT_N, _N], F32, tag="tr", name=f"psa{m}")
                    h = S[m]["h"]
                    for hh in range(HT_N):
                        for d in range(2):
                            nc.tensor.matmul(ps_a[:, hh, :],
                                             h[:, d, hh * 128:(hh + 1) * 128],
                                             S[m]["tm"][:, d, :],
                                             start=(d == 0), stop=(d == 1))
                    S[m]["ps_a"] = ps_a
                for m in range(_MPC):
                    af = work.tile([128, HT_N, _N], FD, tag="af", name=f"af{m}")
                    nc.vector.tensor_copy(out=af, in_=S[m]["ps_a"])
                    S[m]["af"] = af
                for m in range(_MPC):
                    ps_o = ps_sm.tile([_N, _H], F32, tag="sm", name=f"pso{m}")
                    nc.tensor.matmul(ps_o, S[m]["aT1"], woa1_s, start=True, stop=False)
                    nc.tensor.matmul(ps_o, S[m]["aT2"], woa2_s, start=False, stop=False)
                    for hh in range(HT_N):
                        nc.tensor.matmul(ps_o, S[m]["af"][:, hh, :],
                                         woh_s[:, hh, :],
                                         start=False, stop=(hh == HT_N - 1))
                    S[m]["ps_o"] = ps_o
                for m in range(_MPC):
                    o_s = obuf.tile([_N, _H], F32, tag="o", name=f"o{m}")
                    nc.scalar.activation(out=o_s, in_=S[m]["ps_o"], func=RELU)
                    nc.sync.dma_start(out=out[m], in_=o_s)

    nc.compile()
    return nc


def _prep_inputs(atoms, bonds, adj, Wi, Wh, Wo, bo):
    B, N, A = atoms.shape
    EB = bonds.shape[-1]
    H = Wh.shape[0]

    und = []
    for b in range(B):
        vw = np.argwhere(np.triu(adj[b]) > 0)  # canonical (v < w)
        und.append(vw)
    E_max = max(len(e) for e in und)
    E_u = max(32, ((E_max + 31) // 32) * 32)
    assert E_u <= 128, f"E_u={E_u} exceeds one partition tile"

    KX = A + EB  # 147
    META_COLS = 64 + 64 + 2 * E_u + 2 * N + 2 + 2 * E_u + 2 * E_u
    meta = np.zeros((B, 128, META_COLS), np.float32)

    for b in range(B):
        vw = und[b]
        E = len(vw)
        v_e, w_e = vw[:, 0], vw[:, 1]
        deg = adj[b].sum(1)
        ar = np.arange(E)

        atomsT = np.zeros((A + 1, N), np.float32)
        atomsT[:A] = atoms[b].T
        atomsT[A] = 1.0
        # fwd (d=0) = v->w (H[v,w]), bwd (d=1) = w->v (H[w,v])
        # X[:, d, e] = [atoms[src(e,d)] ; bonds(e,d)]  (KX = 133+14 rows)
        X = np.zeros((KX, 2, E_u), np.float32)
        X[:A, 0, :E] = atoms[b, v_e].T
        X[:A, 1, :E] = atoms[b, w_e].T
        X[A:, 0, :E] = bonds[b, v_e, w_e].T
        X[A:, 1, :E] = bonds[b, w_e, v_e].T
        Tfb = np.zeros((E_u, 2, N), np.float32)
        Tfb[ar, 0, w_e] = 1.0  # fwd targets w
        Tfb[ar, 1, v_e] = 1.0  # bwd targets v
        G1T = np.zeros((N, 2, E_u), np.float32)
        G1T[v_e, 0, ar] = 1.0  # fwd source v
        G1T[w_e, 1, ar] = 1.0  # bwd source w
        inv = np.zeros((E_u, 2), np.float32)
        inv[:E, 0] = 1.0 / np.maximum(deg[v_e] - 1.0, 1.0)
        inv[:E, 1] = 1.0 / np.maximum(deg[w_e] - 1.0, 1.0)

        c0 = 0
        meta[b, 0:128, c0:c0 + N] = atomsT[0:128]; c0 += N
        meta[b, 0:A + 1 - 128, c0:c0 + N] = atomsT[128:]; c0 += N
        meta[b, 0:N, c0:c0 + 2 * E_u] = G1T.reshape(N, 2 * E_u); c0 += 2 * E_u
        meta[b, 0:E_u, c0:c0 + 2 * N] = Tfb.reshape(E_u, 2 * N); c0 += 2 * N
        meta[b, 0:E_u, c0:c0 + 2] = inv; c0 += 2
        meta[b, 0:128, c0:c0 + 2 * E_u] = X[0:128].reshape(128, 2 * E_u); c0 += 2 * E_u
        meta[b, 0:KX - 128, c0:c0 + 2 * E_u] = X[128:].reshape(KX - 128, 2 * E_u); c0 += 2 * E_u

    # Wi reordered to match X's row order: [atom rows ; bond rows] = Wi as-is.
    WoA = np.zeros((A + 1, H), np.float32)
    WoA[:A] = Wo[:A]
    WoA[A] = bo
    shared = {
        "Wi": np.ascontiguousarray(Wi),
        "WoA": WoA,
        "Wh": np.ascontiguousarray(Wh),
        "WoH": np.ascontiguousarray(Wo[A:]),
    }

    def shard(x):
        return x.reshape((_NCORES, _MPC) + x.shape[1:])

    per_core = [
        {"meta": shard(meta)[c], **shared}
        for c in range(_NCORES)
    ]
    return per_core, E_u


def kernel(atoms, bonds, adj, Wi, Wh, Wo, bo, _trace=False):
    import sys
    for p in ("/opt/trn_rl_repo",):
        if p not in sys.path:
            sys.path.insert(0, p)
    from concourse.bass_utils import run_bass_kernel_spmd

    atoms = np.asarray(atoms, np.float32)
    bonds = np.asarray(bonds, np.float32)
    adj = np.asarray(adj, np.float32)
    Wi = np.asarray(Wi, np.float32)
    Wh = np.asarray(Wh, np.float32)
    Wo = np.asarray(Wo, np.float32)
    bo = np.asarray(bo, np.float32)

    in_maps, E_u = _prep_inputs(atoms, bonds, adj, Wi, Wh, Wo, bo)

    key = ("nc", E_u, _DTYPE)
    if key not in _cache:
        _cache[key] = _build_nc(E_u, dtype_name=_DTYPE)
    nc = _cache[key]

    res = run_bass_kernel_spmd(nc, in_maps, list(range(_NCORES)), trace=_trace)
    outs = [res.results[c]["out"] for c in range(_NCORES)]
    full = np.concatenate(outs, axis=0).reshape(_B, _N, _H).astype(np.float32)
    if _trace:
        return full, res
    return full



# revision 7
# speedup vs baseline: 1.2589x; 1.2589x over previous
"""DenseMPNN Trainium2 kernel (8-core SPMD, batch data-parallel), v2.

Strategy (v2 = bf16 + DMA/schedule overhaul of the v1 edge-list design):
- Shard batch B=32 across 8 cores (4 molecules/core); replicate weights.
- Host packs each molecule's ~4%-dense adjacency into an UNDIRECTED edge
  list (E_u <= 128): partition row e holds both directions of undirected
  edge {v,w} (fwd = v->w, bwd = w->v).  The reverse-edge lookup in the
  MPNN message is then just the other half of the same row.
    H0[e,d] = relu(X[:,d,e]^T @ Wi)         X = [atoms[src]; bonds]
    iter:  HWh = H @ Wh                     (PE transpose + matmul)
           P  = Tf^T@HWh_f + Tb^T@HWh_b    (= agg @ Wh, [N,H])
           Q_d = G1_d@P - HWh_{1-d}        (= msg_d @ Wh)
           H_d = relu(H0_d + inv_d * Q_d)
    out = relu(atoms@Wo_a + agg_final@Wo_h + bo)
- v2 changes vs v1 (36.9us -> target ~2x):
  * bf16 data everywhere (f32 PSUM accumulate): halves DMA bytes, makes
    small-output matmuls 4x cheaper (f32r pays cpr=4 below 256 free),
    transposes 1.5x cheaper, enables DVE 2x/4x modes on SBUF ops.
  * DMA restructure: 6 large pages instead of 18 tensor loads (HWDGE costs
    ~625ns serialization per DMA), ordered by first-use time.
  * PE warmup: ~40 tiny matmuls on a zeroed tile ramp the PE clock
    (1.54 -> 0.42 ns/cycle over ~3us) during the DMA phase.
  * PSUM->SBUF traffic spread across DVE/Act/Pool; relu on DVE 4x mode.
  * obuf bufs=4 so output stores never block the readout relus.
"""

import numpy as np

_B, _N, _A, _EB, _H = 32, 64, 133, 14, 256
_DEPTH = 3
_NCORES = 8
_MPC = _B // _NCORES  # molecules per core
_KX = _A + _EB  # 147

_cache = {}
_NWARM = 42  # PE clock-ramp warmup matmuls


def _build_nc(E_u, reps=1):
    import sys
    for p in ("/opt/trn_rl_repo",):
        if p not in sys.path:
            sys.path.insert(0, p)
    import concourse.bass as bass  # noqa: F401
    import concourse.mybir as mybir
    import concourse.tile as tile
    from concourse import bacc
    from concourse.masks import make_identity

    BF = mybir.dt.bfloat16
    F32 = mybir.dt.float32
    HT_N = _H // 128  # hidden chunks of 128
    RELU = mybir.ActivationFunctionType.Relu
    MULT = mybir.AluOpType.mult
    ADD = mybir.AluOpType.add

    E2 = 2 * E_u
    XC = 2 * E2 + 2           # X1 | X2 | inv  columns per molecule
    GC = 64 + 64 + E2 + 2 * _N  # aT1 | aT2 | g1 | tm columns per molecule

    nc = bacc.Bacc(None, target_bir_lowering=False, debug=False)

    # --- DRAM I/O (bf16 pages, ordered by first use) ---
    wi_d = nc.dram_tensor("wi", [128, 512], BF, kind="ExternalInput")
    mx_d = nc.dram_tensor("mx", [_MPC, 128, XC], BF, kind="ExternalInput")
    wh_d = nc.dram_tensor("wh", [128, 512], BF, kind="ExternalInput")
    mg_d = nc.dram_tensor("mg", [_MPC, 128, GC], BF, kind="ExternalInput")
    wo_d = nc.dram_tensor("wo", [128, 1024], BF, kind="ExternalInput")
    out_d = nc.dram_tensor("out", [_MPC, _N, _H], F32, kind="ExternalOutput")

    with tile.TileContext(nc) as tc:
        import contextlib
        with contextlib.ExitStack() as ctx:
            consts = ctx.enter_context(tc.tile_pool(name="consts", bufs=1))
            work = ctx.enter_context(tc.tile_pool(name="work", bufs=4))
            hbuf = ctx.enter_context(tc.tile_pool(name="hbuf", bufs=4))
            obuf = ctx.enter_context(tc.tile_pool(name="obuf", bufs=4))
            ps_mm = ctx.enter_context(tc.tile_pool(name="ps_mm", bufs=4, space="PSUM"))
            ps_tr = ctx.enter_context(tc.tile_pool(name="ps_tr", bufs=2, space="PSUM"))
            ps_sm = ctx.enter_context(tc.tile_pool(name="ps_sm", bufs=2, space="PSUM"))

            # ---- PE warmup: ramp the clock while DMAs fly ----
            warm = consts.tile([128, 64], BF)
            nc.vector.memset(warm, 0.0)
            ps_w = ps_tr.tile([64, 64], F32, tag="tr", name="ps_w")
            for i in range(_NWARM):
                nc.tensor.matmul(ps_w, warm, warm[:, 0:64], start=True, stop=True)

            # ---- constants ----
            ident = consts.tile([128, 128], BF)
            make_identity(nc, ident)
            negident = consts.tile([128, 128], BF)
            nc.vector.tensor_scalar_mul(out=negident, in0=ident, scalar1=-1.0)

            wi_s = consts.tile([128, 512], BF)
            nc.sync.dma_start(out=wi_s, in_=wi_d[:, :])
            mxa_s = consts.tile([128, 2, XC], BF)
            nc.sync.dma_start(out=mxa_s, in_=mx_d[0:2].rearrange("m p c -> p m c"))
            mxb_s = consts.tile([128, 2, XC], BF)
            nc.sync.dma_start(out=mxb_s, in_=mx_d[2:4].rearrange("m p c -> p m c"))
            wh_s = consts.tile([128, HT_N, 256], BF)
            nc.sync.dma_start(out=wh_s, in_=wh_d.rearrange("p (c n) -> p c n", c=HT_N))
            mg_s = consts.tile([128, _MPC, GC], BF)
            nc.sync.dma_start(out=mg_s, in_=mg_d.rearrange("m p c -> p m c"))
            wo_s = consts.tile([128, 1024], BF)
            nc.sync.dma_start(out=wo_s, in_=wo_d[:, :])

            wi1 = wi_s[:, 0:256]
            wi2 = wi_s[0:_KX - 128, 256:512]
            woa1 = wo_s[:, 0:256]
            woa2 = wo_s[0:_A + 1 - 128, 256:512]

            def mslice(m):
                mx = mxa_s if m < 2 else mxb_s
                sl = m % 2
                s = {}
                s["X1"] = mx[:, sl, 0:E2].rearrange("p (d e) -> p d e", d=2)
                s["X2"] = mx[0:_KX - 128, sl, E2:2 * E2].rearrange(
                    "p (d e) -> p d e", d=2)
                s["inv"] = mx[0:E_u, sl, 2 * E2:2 * E2 + 2]
                s["aT1"] = mg_s[:, m, 0:64]
                s["aT2"] = mg_s[0:_A + 1 - 128, m, 64:128]
                s["g1"] = mg_s[0:_N, m, 128:128 + E2].rearrange(
                    "p (d e) -> p d e", d=2)
                s["tm"] = mg_s[0:E_u, m, 128 + E2:128 + E2 + 2 * _N].rearrange(
                    "p (d n) -> p d n", d=2)
                return s

            # engine helpers -----------------------------------------------
            def vcopy(eng, out, in_):
                if eng == 0:
                    nc.vector.tensor_copy(out=out, in_=in_)
                elif eng == 1:
                    nc.scalar.copy(out=out, in_=in_)
                else:
                    nc.gpsimd.tensor_copy(out=out, in_=in_)

            def vrelu(eng, out, in_):
                if eng == 0:
                    nc.vector.tensor_scalar_max(out=out, in0=in_, scalar1=0.0)
                elif eng == 1:
                    nc.scalar.activation(out=out, in_=in_, func=RELU)
                else:
                    nc.gpsimd.tensor_scalar_max(out=out, in0=in_, scalar1=0.0)

            def vstt(eng, out, in0, scalar, in1):
                e = nc.vector if eng == 0 else nc.gpsimd
                e.scalar_tensor_tensor(out=out, in0=in0, scalar=scalar,
                                       in1=in1, op0=MULT, op1=ADD)

            for rep in range(reps):
                S = [mslice(m) for m in range(_MPC)]

                # ---- H0 = relu(X^T @ Wi)  [E_u, 2, H] ----
                for m in range(_MPC):
                    ps_h0 = ps_mm.tile([E_u, 2, _H], F32, tag="mm", name=f"psh0{m}")
                    for d in range(2):
                        nc.tensor.matmul(ps_h0[:, d, :], S[m]["X1"][:, d, :], wi1,
                                         start=True, stop=False)
                        nc.tensor.matmul(ps_h0[:, d, :], S[m]["X2"][:, d, :], wi2,
                                         start=False, stop=True)
                    S[m]["ps_h0"] = ps_h0
                # NOTE: GPSIMD/Pool cannot access PSUM — engine 2 is only
                # legal for SBUF->SBUF ops (relu of hn below).
                H0_ENG = [0, 1, 0, 1]
                for m in range(_MPC):
                    h0 = hbuf.tile([E_u, 2, _H], BF, tag="h0", name=f"h0_{m}")
                    vrelu(H0_ENG[m], h0, S[m]["ps_h0"])
                    S[m]["h0"] = h0
                    S[m]["h"] = h0  # initial H == H0 (mask folded into packing)

                # ---- message passing iterations ----
                for it in range(_DEPTH - 1):
                    for m in range(_MPC):
                        ps_t = ps_tr.tile([128, HT_N, 2, E_u], BF, tag="tr",
                                          name=f"pst{m}")
                        h = S[m]["h"]
                        for hh in range(HT_N):
                            for d in range(2):
                                nc.tensor.transpose(
                                    ps_t[:, hh, d, :],
                                    h[:, d, hh * 128:(hh + 1) * 128],
                                    ident[:E_u, :E_u])
                        S[m]["ps_t"] = ps_t
                    for m in range(_MPC):
                        ht = work.tile([128, HT_N, 2, E_u], BF, tag="ht",
                                       name=f"ht{m}")
                        vcopy(0, ht, S[m]["ps_t"])
                        S[m]["ht"] = ht
                    for m in range(_MPC):
                        ps_hw = ps_mm.tile([E_u, 2, _H], F32, tag="mm",
                                           name=f"pshw{m}")
                        for d in range(2):
                            for hh in range(HT_N):
                                nc.tensor.matmul(ps_hw[:, d, :],
                                                 S[m]["ht"][:, hh, d, :],
                                                 wh_s[:, hh, :],
                                                 start=(hh == 0),
                                                 stop=(hh == HT_N - 1))
                        S[m]["ps_hw"] = ps_hw
                    for m in range(_MPC):
                        hwh = work.tile([E_u, 2, _H], BF, tag="hwh", name=f"hwh{m}")
                        vcopy(1, hwh, S[m]["ps_hw"])
                        S[m]["hwh"] = hwh
                    for m in range(_MPC):
                        ps_p = ps_sm.tile([_N, _H], F32, tag="sm", name=f"psp{m}")
                        for d in range(2):
                            nc.tensor.matmul(ps_p, S[m]["tm"][:, d, :],
                                             S[m]["hwh"][:, d, :],
                                             start=(d == 0), stop=(d == 1))
                        S[m]["ps_p"] = ps_p
                    P_ENG = [0, 1, 0, 1]
                    for m in range(_MPC):
                        p_s = work.tile([_N, _H], BF, tag="p", name=f"p{m}")
                        vcopy(P_ENG[m], p_s, S[m]["ps_p"])
                        S[m]["p"] = p_s
                    for m in range(_MPC):
                        ps_q = ps_mm.tile([E_u, 2, _H], F32, tag="mm",
                                          name=f"psq{m}")
                        for d in range(2):
                            nc.tensor.matmul(ps_q[:, d, :], S[m]["g1"][:, d, :],
                                             S[m]["p"], start=True, stop=False)
                            nc.tensor.matmul(ps_q[:, d, :], negident[:E_u, :E_u],
                                             S[m]["hwh"][:, 1 - d, :],
                                             start=False, stop=True)
                        S[m]["ps_q"] = ps_q
                    STT_ENG = [0, 0, 0, 0]
                    for m in range(_MPC):
                        hn_pre = work.tile([E_u, 2, _H], BF, tag="hn_pre",
                                           name=f"hnp{m}")
                        for d in range(2):
                            vstt(STT_ENG[m], hn_pre[:, d, :], S[m]["ps_q"][:, d, :],
                                 S[m]["inv"][:, d:d + 1], S[m]["h0"][:, d, :])
                        S[m]["hn_pre"] = hn_pre
                    for m in range(_MPC):
                        hn = hbuf.tile([E_u, 2, _H], BF, tag="hn", name=f"hn{m}")
                        vrelu(2, hn, S[m]["hn_pre"])
                        S[m]["h"] = hn

                # ---- readout ----
                for m in range(_MPC):
                    ps_a = ps_tr.tile([128, HT_N, _N], F32, tag="tr", name=f"psa{m}")
                    h = S[m]["h"]
                    for hh in range(HT_N):
                        for d in range(2):
                            nc.tensor.matmul(ps_a[:, hh, :],
                                             h[:, d, hh * 128:(hh + 1) * 128],
                                             S[m]["tm"][:, d, :],
                                             start=(d == 0), stop=(d == 1))
                    S[m]["ps_a"] = ps_a
                for m in range(_MPC):
                    af = work.tile([128, HT_N, _N], BF, tag="af", name=f"af{m}")
                    vcopy(0, af, S[m]["ps_a"])
                    S[m]["af"] = af
                for m in range(_MPC):
                    ps_o = ps_mm.tile([_N, _H], F32, tag="mm", name=f"pso{m}")
                    nc.tensor.matmul(ps_o, S[m]["aT1"], woa1, start=True, stop=False)
                    nc.tensor.matmul(ps_o, S[m]["aT2"], woa2, start=False, stop=False)
                    for hh in range(HT_N):
                        nc.tensor.matmul(ps_o, S[m]["af"][:, hh, :],
                                         wo_s[:, 512 + hh * 256:512 + (hh + 1) * 256],
                                         start=False, stop=(hh == HT_N - 1))
                    S[m]["ps_o"] = ps_o
                O_ENG = [0, 1, 0, 1]
                for m in range(_MPC):
                    o_s = obuf.tile([_N, _H], F32, tag="o", name=f"o{m}")
                    vrelu(O_ENG[m], o_s, S[m]["ps_o"])
                    nc.sync.dma_start(out=out_d[m], in_=o_s)

    nc.compile()
    return nc


def _prep_inputs(atoms, bonds, adj, Wi, Wh, Wo, bo):
    import ml_dtypes
    BF = np.dtype(ml_dtypes.bfloat16)
    B, N, A = atoms.shape
    H = Wh.shape[0]

    und = []
    for b in range(B):
        vw = np.argwhere(np.triu(adj[b]) > 0)  # canonical (v < w)
        und.append(vw)
    E_max = max(len(e) for e in und)
    E_u = max(32, ((E_max + 31) // 32) * 32)
    assert E_u <= 128, f"E_u={E_u} exceeds one partition tile"

    E2 = 2 * E_u
    XC = 2 * E2 + 2
    GC = 64 + 64 + E2 + 2 * N
    mx = np.zeros((B, 128, XC), np.float32)
    mg = np.zeros((B, 128, GC), np.float32)

    for b in range(B):
        vw = und[b]
        E = len(vw)
        v_e, w_e = vw[:, 0], vw[:, 1]
        deg = adj[b].sum(1)
        ar = np.arange(E)

        # X[:, d, e] = [atoms[src(e,d)] ; bonds(e,d)]  (KX = 133+14 rows)
        X = np.zeros((_KX, 2, E_u), np.float32)
        X[:A, 0, :E] = atoms[b, v_e].T
        X[:A, 1, :E] = atoms[b, w_e].T
        X[A:, 0, :E] = bonds[b, v_e, w_e].T
        X[A:, 1, :E] = bonds[b, w_e, v_e].T
        inv = np.zeros((E_u, 2), np.float32)
        inv[:E, 0] = 1.0 / np.maximum(deg[v_e] - 1.0, 1.0)
        inv[:E, 1] = 1.0 / np.maximum(deg[w_e] - 1.0, 1.0)
        mx[b, :, 0:E2] = X[0:128].reshape(128, E2)
        mx[b, 0:_KX - 128, E2:2 * E2] = X[128:].reshape(_KX - 128, E2)
        mx[b, 0:E_u, 2 * E2:2 * E2 + 2] = inv

        atomsT = np.zeros((A + 1, N), np.float32)
        atomsT[:A] = atoms[b].T
        atomsT[A] = 1.0
        Tfb = np.zeros((E_u, 2, N), np.float32)
        Tfb[ar, 0, w_e] = 1.0  # fwd targets w
        Tfb[ar, 1, v_e] = 1.0  # bwd targets v
        G1T = np.zeros((N, 2, E_u), np.float32)
        G1T[v_e, 0, ar] = 1.0  # fwd source v
        G1T[w_e, 1, ar] = 1.0  # bwd source w
        mg[b, 0:128, 0:64] = atomsT[0:128]
        mg[b, 0:A + 1 - 128, 64:128] = atomsT[128:]
        mg[b, 0:N, 128:128 + E2] = G1T.reshape(N, E2)
        mg[b, 0:E_u, 128 + E2:128 + E2 + 2 * N] = Tfb.reshape(E_u, 2 * N)

    wi = np.zeros((128, 512), np.float32)
    wi[:, 0:256] = Wi[0:128]
    wi[0:_KX - 128, 256:512] = Wi[128:]
    wh = Wh.reshape(2, 128, 256).transpose(1, 0, 2).reshape(128, 512)
    wo = np.zeros((128, 1024), np.float32)
    wo[:, 0:256] = Wo[0:128]
    wo[0:A + 1 - 128, 256:512] = np.concatenate([Wo[128:A], bo[None, :]], axis=0)
    wo[:, 512:1024] = Wo[A:].reshape(2, 128, 256).transpose(1, 0, 2).reshape(128, 512)

    shared = {
        "wi": wi.astype(BF),
        "wh": np.ascontiguousarray(wh).astype(BF),
        "wo": wo.astype(BF),
    }

    def shard(x):
        return x.reshape((_NCORES, _MPC) + x.shape[1:])

    mx8, mg8 = shard(mx.astype(BF)), shard(mg.astype(BF))
    per_core = [
        {"mx": mx8[c], "mg": mg8[c], **shared}
        for c in range(_NCORES)
    ]
    return per_core, E_u


def kernel(atoms, bonds, adj, Wi, Wh, Wo, bo, _trace=False):
    import sys
    for p in ("/opt/trn_rl_repo",):
        if p not in sys.path:
            sys.path.insert(0, p)
    from concourse.bass_utils import run_bass_kernel_spmd

    atoms = np.asarray(atoms, np.float32)
    bonds = np.asarray(bonds, np.float32)
    adj = np.asarray(adj, np.float32)
    Wi = np.asarray(Wi, np.float32)
    Wh = np.asarray(Wh, np.float32)
    Wo = np.asarray(Wo, np.float32)
    bo = np.asarray(bo, np.float32)

    in_maps, E_u = _prep_inputs(atoms, bonds, adj, Wi, Wh, Wo, bo)

    key = ("nc", E_u)
    if key not in _cache:
        _cache[key] = _build_nc(E_u)
    nc = _cache[key]

    res = run_bass_kernel_spmd(nc, in_maps, list(range(_NCORES)), trace=_trace)
    outs = [res.results[c]["out"] for c in range(_NCORES)]
    full = np.concatenate(outs, axis=0).reshape(_B, _N, _H).astype(np.float32)
    if _trace:
        return full, res
    return full


# revision 8
# speedup vs baseline: 1.5507x; 1.2318x over previous
"""DenseMPNN Trainium2 kernel (8-core SPMD, batch data-parallel), v3.

Strategy:
- Shard batch B=32 across 8 cores (4 molecules/core); replicate weights.
- Host packs each molecule's ~4%-dense adjacency into an UNDIRECTED edge
  list (E_u <= 128): partition row e holds both directions of undirected
  edge {v,w} (fwd = v->w, bwd = w->v).
    H0[e,d] = relu(X[:,d,e]^T @ Wi)          X = [atoms[src]; bonds]
    iter:  HWh_d = H_d @ Wh                  (PE transpose + matmul)
           Q_d  = M_d0@HWh_0 + M_d1@HWh_1 + I@H0_d
           H_d  = relu(Q_d)
    out = relu(atoms@Wo_a + agg_final@Wo_h + bo)
  where M_de = inv_d (.) (G1_d @ T_e^T) - [e==1-d] diag(inv_d) are
  host-built [E,E] edge->edge message matrices: they fold the node
  aggregation (T), the source gather (G1), the reverse-edge subtraction
  and the 1/n_nbr scaling into ONE stationary operand, so the whole
  per-iteration update is matmuls + a single relu. This removes the
  [N,H] P round trip and the DVE-only scalar_tensor_tensor that
  bottlenecked v2 (Pool cannot access PSUM; Act has no stt).
- bf16 data (f32 PSUM accumulate); PE clock warmed up by dummy matmuls
  during the DMA phase; Wi loaded via Pool/SWDGE in parallel with the
  SP/HWDGE page loads; PSUM->SBUF copies split across DVE and Act.
"""

import numpy as np

_B, _N, _A, _EB, _H = 32, 64, 133, 14, 256
_DEPTH = 3
_NCORES = 8
_MPC = _B // _NCORES  # molecules per core
_KX = _A + _EB  # 147

_cache = {}
_NWARM = 48  # PE clock-ramp warmup matmuls


def _build_nc(E_u, reps=1):
    import sys
    for p in ("/opt/trn_rl_repo",):
        if p not in sys.path:
            sys.path.insert(0, p)
    import concourse.bass as bass  # noqa: F401
    import concourse.mybir as mybir
    import concourse.tile as tile
    from concourse import bacc
    from concourse.masks import make_identity

    BF = mybir.dt.bfloat16
    F32 = mybir.dt.float32
    HT_N = _H // 128  # hidden chunks of 128
    RELU = mybir.ActivationFunctionType.Relu

    E2 = 2 * E_u
    XC = 2 * E2 + 2                 # X1 | X2 | inv  columns per molecule
    GC = 64 + 64 + 2 * _N + 4 * E_u  # aT1 | aT2 | tm | M  columns per molecule

    nc = bacc.Bacc(None, target_bir_lowering=False, debug=False)

    # --- DRAM I/O (bf16 pages, ordered by first use) ---
    mx_d = nc.dram_tensor("mx", [_MPC, 128, XC], BF, kind="ExternalInput")
    wi_d = nc.dram_tensor("wi", [128, 512], BF, kind="ExternalInput")
    wh_d = nc.dram_tensor("wh", [128, 512], BF, kind="ExternalInput")
    mg_d = nc.dram_tensor("mg", [_MPC, 128, GC], BF, kind="ExternalInput")
    wo_d = nc.dram_tensor("wo", [128, 1024], BF, kind="ExternalInput")
    out_d = nc.dram_tensor("out", [_MPC, _N, _H], F32, kind="ExternalOutput")

    with tile.TileContext(nc) as tc:
        import contextlib
        with contextlib.ExitStack() as ctx:
            consts = ctx.enter_context(tc.tile_pool(name="consts", bufs=1))
            work = ctx.enter_context(tc.tile_pool(name="work", bufs=4))
            hbuf = ctx.enter_context(tc.tile_pool(name="hbuf", bufs=4))
            ps_mm = ctx.enter_context(tc.tile_pool(name="ps_mm", bufs=5, space="PSUM"))
            ps_tr = ctx.enter_context(tc.tile_pool(name="ps_tr", bufs=2, space="PSUM"))

            # ---- PE warmup (independent of all loads): ramp the PE clock
            # from 0.65 GHz to 2.4 GHz while the DMAs fly. ----
            warm = consts.tile([128, 64], BF)
            nc.vector.memset(warm, 0.0)
            ps_w = ps_tr.tile([64, 64], F32, tag="tr", name="ps_w")
            for i in range(_NWARM):
                nc.tensor.matmul(ps_w, warm, warm[:, 0:64], start=True, stop=True)

            # ---- loads: Wi via Pool/SWDGE (parallel issue path), pages via
            # SP/HWDGE in first-use order. ----
            wi_s = consts.tile([128, 512], BF)
            nc.gpsimd.dma_start(out=wi_s, in_=wi_d[:, :])
            mxa_s = consts.tile([128, 2, XC], BF)
            nc.sync.dma_start(out=mxa_s, in_=mx_d[0:2].rearrange("m p c -> p m c"))
            mxb_s = consts.tile([128, 2, XC], BF)
            nc.sync.dma_start(out=mxb_s, in_=mx_d[2:4].rearrange("m p c -> p m c"))
            wh_s = consts.tile([128, HT_N, 256], BF)
            nc.sync.dma_start(out=wh_s, in_=wh_d.rearrange("p (c n) -> p c n", c=HT_N))
            mg_s = consts.tile([128, _MPC, GC], BF)
            nc.sync.dma_start(out=mg_s, in_=mg_d.rearrange("m p c -> p m c"))
            wo_s = consts.tile([128, 1024], BF)
            nc.sync.dma_start(out=wo_s, in_=wo_d[:, :])

            # ---- small consts (Pool, after its SWDGE issue) ----
            ident = consts.tile([128, 128], BF)
            make_identity(nc, ident)

            wi1 = wi_s[:, 0:256]
            wi2 = wi_s[0:_KX - 128, 256:512]
            woa1 = wo_s[:, 0:256]
            woa2 = wo_s[0:_A + 1 - 128, 256:512]

            def mslice(m):
                mx = mxa_s if m < 2 else mxb_s
                sl = m % 2
                s = {}
                s["X1"] = mx[:, sl, 0:E2].rearrange("p (d e) -> p d e", d=2)
                s["X2"] = mx[0:_KX - 128, sl, E2:2 * E2].rearrange(
                    "p (d e) -> p d e", d=2)
                s["aT1"] = mg_s[:, m, 0:64]
                s["aT2"] = mg_s[0:_A + 1 - 128, m, 64:128]
                s["tm"] = mg_s[0:E_u, m, 128:128 + 2 * _N].rearrange(
                    "p (d n) -> p d n", d=2)
                s["M"] = mg_s[0:E_u, m, 128 + 2 * _N:GC].rearrange(
                    "p (j e) -> p j e", j=4)  # j = 2*d + e
                return s

            def vrelu(eng, out, in_):
                if eng == 0:
                    nc.vector.tensor_scalar_max(out=out, in0=in_, scalar1=0.0)
                else:
                    nc.scalar.activation(out=out, in_=in_, func=RELU)

            for rep in range(reps):
                S = [mslice(m) for m in range(_MPC)]

                # ---- H0 = relu(X^T @ Wi)  [E_u, 2, H] ----
                for m in range(_MPC):
                    ps_h0 = ps_mm.tile([E_u, 2, _H], F32, tag="mm", name=f"psh0{m}")
                    for d in range(2):
                        nc.tensor.matmul(ps_h0[:, d, :], S[m]["X1"][:, d, :], wi1,
                                         start=True, stop=False)
                        nc.tensor.matmul(ps_h0[:, d, :], S[m]["X2"][:, d, :], wi2,
                                         start=False, stop=True)
                    S[m]["ps_h0"] = ps_h0
                for m in range(_MPC):
                    h0 = hbuf.tile([E_u, 2, _H], BF, tag="h0", name=f"h0_{m}")
                    vrelu(m % 2, h0, S[m]["ps_h0"])
                    S[m]["h0"] = h0
                    S[m]["h"] = h0  # initial H == H0 (mask folded into packing)

                # ---- message passing iterations ----
                for it in range(_DEPTH - 1):
                    for m in range(_MPC):
                        ps_t = ps_tr.tile([128, HT_N, 2, E_u], BF, tag="tr",
                                          name=f"pst{m}")
                        h = S[m]["h"]
                        for hh in range(HT_N):
                            for d in range(2):
                                nc.tensor.transpose(
                                    ps_t[:, hh, d, :],
                                    h[:, d, hh * 128:(hh + 1) * 128],
                                    ident[:E_u, :E_u])
                        S[m]["ps_t"] = ps_t
                    for m in range(_MPC):
                        ht = work.tile([128, HT_N, 2, E_u], BF, tag="ht",
                                       name=f"ht{m}")
                        nc.vector.tensor_copy(out=ht, in_=S[m]["ps_t"])
                        S[m]["ht"] = ht
                    for m in range(_MPC):
                        ps_hw = ps_mm.tile([E_u, 2, _H], F32, tag="mm",
                                           name=f"pshw{m}")
                        for d in range(2):
                            for hh in range(HT_N):
                                nc.tensor.matmul(ps_hw[:, d, :],
                                                 S[m]["ht"][:, hh, d, :],
                                                 wh_s[:, hh, :],
                                                 start=(hh == 0),
                                                 stop=(hh == HT_N - 1))
                        S[m]["ps_hw"] = ps_hw
                    for m in range(_MPC):
                        hwh = work.tile([E_u, 2, _H], BF, tag="hwh", name=f"hwh{m}")
                        nc.scalar.copy(out=hwh, in_=S[m]["ps_hw"])
                        S[m]["hwh"] = hwh
                    # Q_d = M_d0 @ HWh_0 + M_d1 @ HWh_1 + H0_d  (one PSUM group)
                    for m in range(_MPC):
                        ps_q = ps_mm.tile([E_u, 2, _H], F32, tag="mm",
                                          name=f"psq{m}")
                        nc.tensor.matmul(ps_q, ident[:E_u, :E_u],
                                         S[m]["h0"].rearrange("e d h -> e (d h)"),
                                         start=True, stop=False,
                                         skip_group_check=True)
                        for d in range(2):
                            for e in range(2):
                                nc.tensor.matmul(ps_q[:, d, :],
                                                 S[m]["M"][:, 2 * d + e, :],
                                                 S[m]["hwh"][:, e, :],
                                                 start=False,
                                                 stop=(d == 1 and e == 1),
                                                 skip_group_check=True)
                        S[m]["ps_q"] = ps_q
                    for m in range(_MPC):
                        hn = hbuf.tile([E_u, 2, _H], BF, tag="hn", name=f"hn{m}")
                        vrelu(m % 2, hn, S[m]["ps_q"])
                        S[m]["h"] = hn

                # ---- readout ----
                for m in range(_MPC):
                    ps_a = ps_tr.tile([128, HT_N, _N], F32, tag="tr", name=f"psa{m}")
                    h = S[m]["h"]
                    for hh in range(HT_N):
                        for d in range(2):
                            nc.tensor.matmul(ps_a[:, hh, :],
                                             h[:, d, hh * 128:(hh + 1) * 128],
                                             S[m]["tm"][:, d, :],
                                             start=(d == 0), stop=(d == 1))
                    S[m]["ps_a"] = ps_a
                for m in range(_MPC):
                    af = work.tile([128, HT_N, _N], BF, tag="af", name=f"af{m}")
                    nc.vector.tensor_copy(out=af, in_=S[m]["ps_a"])
                    S[m]["af"] = af
                o_all = consts.tile([_N, _MPC, _H], F32, name="o_all")
                for m in range(_MPC):
                    ps_o = ps_mm.tile([_N, _H], F32, tag="mm", name=f"pso{m}")
                    nc.tensor.matmul(ps_o, S[m]["aT1"], woa1, start=True, stop=False)
                    nc.tensor.matmul(ps_o, S[m]["aT2"], woa2, start=False, stop=False)
                    for hh in range(HT_N):
                        nc.tensor.matmul(ps_o, S[m]["af"][:, hh, :],
                                         wo_s[:, 512 + hh * 256:512 + (hh + 1) * 256],
                                         start=False, stop=(hh == HT_N - 1))
                    S[m]["ps_o"] = ps_o
                for m in range(_MPC):
                    vrelu(m % 2, o_all[:, m, :], S[m]["ps_o"])
                # single merged store: [N, MPC, H] sbuf -> [MPC, N, H] dram
                nc.sync.dma_start(out=out_d.rearrange("m n h -> n m h"), in_=o_all)

    nc.compile()
    return nc


def _prep_inputs(atoms, bonds, adj, Wi, Wh, Wo, bo):
    import ml_dtypes
    BF = np.dtype(ml_dtypes.bfloat16)
    B, N, A = atoms.shape
    H = Wh.shape[0]

    und = []
    for b in range(B):
        vw = np.argwhere(np.triu(adj[b]) > 0)  # canonical (v < w)
        und.append(vw)
    E_max = max(len(e) for e in und)
    E_u = max(32, ((E_max + 31) // 32) * 32)
    assert E_u <= 128, f"E_u={E_u} exceeds one partition tile"

    E2 = 2 * E_u
    XC = 2 * E2 + 2
    GC = 64 + 64 + 2 * N + 4 * E_u
    mx = np.zeros((B, 128, XC), np.float32)
    mg = np.zeros((B, 128, GC), np.float32)

    for b in range(B):
        vw = und[b]
        E = len(vw)
        v_e, w_e = vw[:, 0], vw[:, 1]
        deg = adj[b].sum(1)
        ar = np.arange(E)

        # X[:, d, e] = [atoms[src(e,d)] ; bonds(e,d)]  (KX = 133+14 rows)
        X = np.zeros((_KX, 2, E_u), np.float32)
        X[:A, 0, :E] = atoms[b, v_e].T
        X[:A, 1, :E] = atoms[b, w_e].T
        X[A:, 0, :E] = bonds[b, v_e, w_e].T
        X[A:, 1, :E] = bonds[b, w_e, v_e].T
        mx[b, :, 0:E2] = X[0:128].reshape(128, E2)
        mx[b, 0:_KX - 128, E2:2 * E2] = X[128:].reshape(_KX - 128, E2)
        inv = np.zeros((E_u, 2), np.float32)
        inv[:E, 0] = 1.0 / np.maximum(deg[v_e] - 1.0, 1.0)
        inv[:E, 1] = 1.0 / np.maximum(deg[w_e] - 1.0, 1.0)
        mx[b, 0:E_u, 2 * E2:2 * E2 + 2] = inv  # kept for reference/debug

        atomsT = np.zeros((A + 1, N), np.float32)
        atomsT[:A] = atoms[b].T
        atomsT[A] = 1.0
        src = np.zeros((2, E_u), np.int64)  # src node of edge (d, e)
        tgt = np.zeros((2, E_u), np.int64)  # tgt node of edge (d, e)
        src[0, :E], src[1, :E] = v_e, w_e
        tgt[0, :E], tgt[1, :E] = w_e, v_e
        Tfb = np.zeros((E_u, 2, N), np.float32)
        Tfb[ar, 0, w_e] = 1.0
        Tfb[ar, 1, v_e] = 1.0
        # M_de[e1,e2] = inv_d[e1] * [src_d(e1) == tgt_e(e2)]
        #   - [e == 1-d] inv_d[e1] * [e1 == e2]
        # stored transposed (lhsT layout): band[:, 2d+e, :][e2, e1] = M_de[e1, e2]
        Mband = np.zeros((E_u, 4, E_u), np.float32)
        for d in range(2):
            for e in range(2):
                Mde = (src[d][:, None] == tgt[e][None, :]).astype(np.float32)
                if E < E_u:
                    Mde[E:, :] = 0.0
                    Mde[:, E:] = 0.0
                Mde *= inv[:, d][:, None]
                if e == 1 - d:
                    Mde -= np.diag(inv[:, d])
                Mband[:, 2 * d + e, :] = Mde.T
        mg[b, 0:128, 0:64] = atomsT[0:128]
        mg[b, 0:A + 1 - 128, 64:128] = atomsT[128:]
        mg[b, 0:E_u, 128:128 + 2 * N] = Tfb.reshape(E_u, 2 * N)
        mg[b, 0:E_u, 128 + 2 * N:GC] = Mband.reshape(E_u, 4 * E_u)

    wi = np.zeros((128, 512), np.float32)
    wi[:, 0:256] = Wi[0:128]
    wi[0:_KX - 128, 256:512] = Wi[128:]
    wh = Wh.reshape(2, 128, 256).transpose(1, 0, 2).reshape(128, 512)
    wo = np.zeros((128, 1024), np.float32)
    wo[:, 0:256] = Wo[0:128]
    wo[0:A + 1 - 128, 256:512] = np.concatenate([Wo[128:A], bo[None, :]], axis=0)
    wo[:, 512:1024] = Wo[A:].reshape(2, 128, 256).transpose(1, 0, 2).reshape(128, 512)

    shared = {
        "wi": wi.astype(BF),
        "wh": np.ascontiguousarray(wh).astype(BF),
        "wo": wo.astype(BF),
    }

    def shard(x):
        return x.reshape((_NCORES, _MPC) + x.shape[1:])

    mx8, mg8 = shard(mx.astype(BF)), shard(mg.astype(BF))
    per_core = [
        {"mx": mx8[c], "mg": mg8[c], **shared}
        for c in range(_NCORES)
    ]
    return per_core, E_u


def kernel(atoms, bonds, adj, Wi, Wh, Wo, bo, _trace=False):
    import sys
    for p in ("/opt/trn_rl_repo",):
        if p not in sys.path:
            sys.path.insert(0, p)
    from concourse.bass_utils import run_bass_kernel_spmd

    atoms = np.asarray(atoms, np.float32)
    bonds = np.asarray(bonds, np.float32)
    adj = np.asarray(adj, np.float32)
    Wi = np.asarray(Wi, np.float32)
    Wh = np.asarray(Wh, np.float32)
    Wo = np.asarray(Wo, np.float32)
    bo = np.asarray(bo, np.float32)

    in_maps, E_u = _prep_inputs(atoms, bonds, adj, Wi, Wh, Wo, bo)

    key = ("nc", E_u)
    if key not in _cache:
        _cache[key] = _build_nc(E_u)
    nc = _cache[key]

    res = run_bass_kernel_spmd(nc, in_maps, list(range(_NCORES)), trace=_trace)
    outs = [res.results[c]["out"] for c in range(_NCORES)]
    full = np.concatenate(outs, axis=0).reshape(_B, _N, _H).astype(np.float32)
    if _trace:
        return full, res
    return full
